# revision 15
# baseline (speedup 1.0000x reference)
"""HSIC loss kernel for Trainium2, 8 NeuronCores.

Math: for each feature column c of X [2048, 16], K_c = rbf kernel matrix
(zero diag). Output = sum over feature pairs a<b of squared unbiased-HSIC
combination of T[a,b]=sum(K_a*K_b), rowsums A, total sums S.

Device strategy (SPMD, symmetric-triangle sharding, all-f32 precision):
  Block triangle coverage: core r owns j-blocks jb=r (i-span 1152) and
  jb=r+8 (i-span 1024); host rotates each core's input planes by 128*r so
  every core compiles the SAME instruction stream.
     K' = Derivative_Erf(y_i - y_j),  y = fp16(sqrt(g_c) * x)
  One ACT per (block, phase, c) with the phase i-span as free dim, f32r
  output into a blocked [(i//8), c, i%8] ktile (full f32 precision).
  f32r matmuls hit the 1-cycle/column mode only at moving dim 256, so
  gram matmuls use 2-chunk rhs strips [128,256] with even/odd PSUM
  accumulator classes (left/right output halves are the useful ones),
  and column-sum matmuls use the same 256-strips with a sliding one-hot
  [128,7] weight slice so 4 consecutive strips land on PSUM partitions
  0..3 of one tile and drain with one cheap 4-partition DVE copy.
  Rowsums: ACT accum_out on one phase (keeps Scalar near pure-ACTIVATE
  occupancy) + DVE 4D-AP reduces for the rest.  Host assembles A, T in
  float64 and finishes the tiny [16,16] HSIC.
"""

import sys
import numpy as np

if "/opt/trn_rl_repo" not in sys.path:
    sys.path.insert(0, "/opt/trn_rl_repo")

N = 2048
D = 16
P = 128
NCORES = 8
GJ = 8                    # i's packed per gram chunk col-group -> D*GJ = 128
CH = D * GJ               # 128
S0, S1 = 1152, 1024       # i-spans of block 0 (jb=r) and block 1 (jb=r+8)
OFF1 = 1024               # plane offset of block 1
PHASES = ((0, 384), (384, 384), (768, 384), (0, 384), (384, 320), (704, 320))
PH_BLK = (0, 0, 0, 1, 1, 1)
ACC_PH = ()               # phases whose rowsums come from ACT accum_out
NPH = len(PHASES)
SQPI = float(np.sqrt(np.pi))
C32 = np.float32(2.0 / np.sqrt(np.pi))      # DErf(0) table value on device
NCSROW = (S0 // 16 - 8) + (S1 // 16 - 8)    # 120 off-diag colsum strips

_NC_CACHE = {}


def _patch_tile_drain():
    """Walrus in this container accepts only 1 sync-wait per instruction.
    Tile routinely attaches several. Hoist extra waits onto single-wait NoOp
    carriers emitted just before the instruction on the same engine, and
    split the tail drain's per-engine waits the same way."""
    import concourse.mybir as mybir
    import concourse.tile as tile_mod
    from concourse.vector_clock import ScopedClock, VectorClock

    if getattr(tile_mod.TileContext, "_drain_patched", False):
        return

    orig_add = tile_mod.TileContext._add_instruction
    counter = [0]

    def _add_instruction(self, inst):
        si = inst.sync_info
        if si is not None and si.on_wait is not None and len(si.on_wait) > 1:
            waits = list(si.on_wait)
            for w in waits[:-1]:
                counter[0] += 1
                carrier = mybir.InstNoOp(name=f"waitc-{counter[0]}")
                carrier.engine = inst.engine
                carrier.sync_info = mybir.SyncInfo(on_wait=[w], on_update=[])
                orig_add(self, carrier)
            inst.sync_info = mybir.SyncInfo(
                on_wait=[waits[-1]], on_update=list(si.on_update or [])
            )
        orig_add(self, inst)

    def _drain_and_barrier(self, tick_clock, wait_clock):
        vec = list(tick_clock.global_clock)
        for i, v in enumerate(vec):
            if v <= 0:
                continue
            sub = [v if j == i else 0 for j in range(len(vec))]
            carrier = self.nc.sync.nop(nofuse=True)
            wait_clock.add_sem_waits(
                carrier.ins, ScopedClock({None: VectorClock(sub)})
            )
        self.nc.sync.drain()
        self.nc.all_engine_barrier()
        popped = self.nc._tile_sem_poison_stack.pop()
        assert popped is self._sem_poison
        self.nc.clear_and_free_semaphores(list(self.sems.allocated().values()))
        self.nc.all_engine_barrier()

    tile_mod.TileContext._add_instruction = _add_instruction
    tile_mod.TileContext._drain_and_barrier = _drain_and_barrier
    tile_mod.TileContext._drain_patched = True


def _build_nc():
    import concourse.bass as bass
    import concourse.mybir as mybir
    from concourse.tile import TileContext

    _patch_tile_drain()

    f32 = mybir.dt.float32
    f32r = mybir.dt.float32r
    f16 = mybir.dt.float16

    nc = bass.Bass("TRN2")
    xrep_d = nc.dram_tensor("xrep", [P, D * (S0 + S1)], f16, kind="ExternalInput")
    bias_d = nc.dram_tensor("bias_t", [P, 2 * D], f32, kind="ExternalInput")
    z_d = nc.dram_tensor("z_t", [P, 63], f32r, kind="ExternalInput")
    rs_d = nc.dram_tensor("rsums", [P, NPH * D], f32, kind="ExternalOutput")
    gram_d = nc.dram_tensor("gram", [P, 4 * 256], f32, kind="ExternalOutput")
    cs_d = nc.dram_tensor("cs", [NCSROW, 256], f32, kind="ExternalOutput")

    # gram accumulation counts per class (diag/offd x even/odd g)
    total_uses = [0, 0, 0, 0]
    for S in (S0, S1):
        for g in range(S // GJ):
            total_uses[(0 if g < 16 else 2) + (g % 2)] += 1

    with TileContext(nc) as tc:
        with (
            tc.tile_pool(name="const", bufs=1) as cpool,
            tc.tile_pool(name="xp", bufs=6) as xpool,
            tc.tile_pool(name="csb", bufs=2) as csbpool,
            tc.tile_pool(name="gps", bufs=1, space="PSUM") as gpool,
            tc.tile_pool(name="cps", bufs=3, space="PSUM") as cspool,
        ):
            bias_sb = cpool.tile([P, 2 * D], f32)
            zt = cpool.tile([P, 63], f32r)
            rsums = cpool.tile([P, NPH * D], f32)
            scratch = cpool.tile([P, 1], f32)
            gsb = cpool.tile([P, 4 * 256], f32)
            kt0 = cpool.tile([P, S0 * D], f32r)
            kt1 = cpool.tile([P, S1 * D], f32r)

            # dummy ACT on a framework const AP: pulls the Derivative_Erf
            # table load forward so it overlaps the first input DMAs
            nc.scalar.activation(
                out=scratch[:],
                in_=nc.const_aps.tensor(0.0, (P, 1), f32),
                func=mybir.ActivationFunctionType.Derivative_Erf,
            )
            nc.sync.dma_start(bias_sb[:], bias_d[:])

            grams = [gpool.tile([P, 256], f32, name=f"gram{t}") for t in range(4)]
            use_ct = [0, 0, 0, 0]
            # colsum psum uses: (n_strips, cs_d row base) per 32-strip group
            cs_uses = [(32, 0), (32, 32), (32, 64), (24, 96)]
            cs_use = 0
            cs_r = 0
            cs_tile = None

            def drain_grams(ts):
                for t in ts:
                    nc.vector.tensor_copy(
                        gsb[:, t * 256 : (t + 1) * 256], grams[t][:]
                    )

            wps = gpool.tile([1, 512], f32, name="warm")

            xoff = 0             # running DRAM column offset of xrep
            for ph, (g0i, Si) in enumerate(PHASES):
                blk = PH_BLK[ph]
                kt = kt0 if blk == 0 else kt1
                k4 = kt[:].rearrange("p (g c i) -> p g c i", c=D, i=GJ)
                g0 = g0i // GJ
                Gh = Si // GJ
                # input: batched DMAs of feature-strips. Phase 0 splits the
                # first batch in half so the first ACT starts sooner.
                batches = [2, 2, 4, 4, 4] if ph == 0 else [4, 4, 4, 4]
                c = 0
                for bi, nb in enumerate(batches):
                    xq = xpool.tile([P, nb * Si], f16, name="xq")
                    nc.sync.dma_start(
                        xq[:], xrep_d[:, xoff : xoff + nb * Si]
                    )
                    xoff += nb * Si
                    if ph == 0 and bi == 0:
                        # zt load can trail the first input batch
                        nc.sync.dma_start(zt[:], z_d[:])
                        # dummy f16 matmuls: keep the PE HAM window busy
                        # during phase-0 ACTs so real matmuls start warm
                        for _ in range(26):
                            nc.tensor.matmul(
                                wps[:],
                                lhsT=xq[:, 0:1],
                                rhs=xq[:, 0:512],
                                start=True,
                                stop=True,
                            )
                    for k in range(nb):
                        acc = (
                            rsums[:, ph * D + c : ph * D + c + 1]
                            if ph in ACC_PH
                            else None
                        )
                        nc.scalar.activation(
                            out=k4[:, g0 : g0 + Gh, c, :],
                            in_=xq[:, k * Si : (k + 1) * Si],
                            func=mybir.ActivationFunctionType.Derivative_Erf,
                            bias=bias_sb[:, blk * D + c : blk * D + c + 1],
                            accum_out=acc,
                        )
                        c += 1
                if ph not in ACC_PH:
                    # rowsums for this phase on DVE (4D AP, reduce over g,i)
                    k4c = kt[:].rearrange("p (g c i) -> p c g i", c=D, i=GJ)
                    nc.vector.tensor_reduce(
                        out=rsums[:, ph * D : (ph + 1) * D],
                        in_=k4c[:, :, g0 : g0 + Gh, :],
                        axis=mybir.AxisListType.XY,
                        op=mybir.AluOpType.add,
                    )
                # grams + colsums per 2-chunk 256-col strip.  In the last
                # phase emit all colsums first so the final cs drain + DMA
                # overlap the remaining gram matmuls instead of tailing.
                def emit_gram(s):
                    rhs = kt[:, s * 256 : (s + 1) * 256]
                    for g in (2 * s, 2 * s + 1):
                        t = (0 if g < 16 else 2) + (g % 2)
                        nc.tensor.matmul(
                            grams[t][:],
                            lhsT=kt[:, g * CH : (g + 1) * CH],
                            rhs=rhs,
                            start=(use_ct[t] == 0),
                            stop=(use_ct[t] == total_uses[t] - 1),
                        )
                        use_ct[t] += 1
                        if use_ct[t] == total_uses[t] and t == 1:
                            drain_grams((0, 1))   # diag classes done early
                            nc.sync.dma_start(
                                gram_d[:, 0:512], gsb[:, 0:512]
                            )

                def emit_colsum(s):
                    nonlocal cs_use, cs_r, cs_tile
                    if s < 8:
                        return  # diag strip: colsums not needed
                    nuse, rowbase = cs_uses[cs_use]
                    r = cs_r
                    if r == 0:
                        cs_tile = cspool.tile([32, 256], f32, name="cs")
                    # lhsT = zt[:, 31-r : 63-r] puts the ones column at row r
                    nc.tensor.matmul(
                        cs_tile[0:32, :],
                        lhsT=zt[:, 31 - r : 63 - r],
                        rhs=kt[:, s * 256 : (s + 1) * 256],
                        start=(r == 0),
                        stop=(r == nuse - 1),
                    )
                    cs_r += 1
                    if cs_r == nuse:
                        cs_r = 0
                        cs_use += 1
                        stage = csbpool.tile([32, 256], f32, name="csb")
                        nc.vector.tensor_copy(
                            stage[0:nuse, :], cs_tile[0:nuse, :]
                        )
                        nc.sync.dma_start(
                            cs_d[rowbase : rowbase + nuse, :], stage[0:nuse, :]
                        )

                strips = range(g0i // 16, (g0i + Si) // 16)
                if ph == NPH - 1:
                    for s in strips:
                        emit_colsum(s)
                    for s in strips:
                        emit_gram(s)
                else:
                    for s in strips:
                        emit_gram(s)
                        emit_colsum(s)

            nc.sync.dma_start(rs_d[:], rsums[:])
            drain_grams((2, 3))
            nc.sync.dma_start(gram_d[:, 512:], gsb[:, 512:])
    return nc


def _get_nc():
    if "nc" not in _NC_CACHE:
        _NC_CACHE["nc"] = _build_nc()
    return _NC_CACHE["nc"]


def _make_in_maps(X):
    Xd = X.astype(np.float64)
    meanD = 2.0 * (np.mean(Xd * Xd, axis=0) - np.mean(Xd, axis=0) ** 2)  # [D]
    g = 1.0 / (2.0 * meanD)                # gamma = 1/(2*sigma^2)
    s = np.sqrt(g).astype(np.float32)      # sqrt(gamma) per column

    # Device sees fp16-rounded prescaled samples y = fp16(s_c * x); build the
    # bias from the same rounded values so the kernel diagonal is DErf(0).
    Y16 = (X.astype(np.float32) * s[None, :]).astype(np.float16)   # [N, D]

    # one-hot weight for colsum matmuls: col 31 ones, rest zero
    z_t = np.zeros((P, 63), np.float32)
    z_t[:, 31] = 1.0

    in_maps = []
    for r in range(NCORES):
        cols = []
        for ph, (g0i, Si) in enumerate(PHASES):
            blk = PH_BLK[ph]
            base = 128 * r + (OFF1 if blk else 0) + g0i
            idx = (base + np.arange(Si)) % N
            for c in range(D):
                cols.append(Y16[idx, c])
        xrow = np.concatenate(cols)                       # [D*(S0+S1)] f16
        xrep = np.ascontiguousarray(
            np.broadcast_to(xrow[None, :], (P, xrow.shape[0]))
        )
        bias = np.empty((P, 2 * D), np.float32)
        for blk, jb in enumerate((r, r + 8)):
            yj = Y16[jb * P : (jb + 1) * P, :].astype(np.float32)  # [P, D]
            bias[:, blk * D : (blk + 1) * D] = -yj
        in_maps.append({"xrep": xrep, "bias_t": bias, "z_t": z_t})
    return in_maps


def _combine(results):
    # Device K' = (2/sqrt(pi)) * K, exact f32 everywhere.
    Ap = np.zeros((D, N), dtype=np.float64)   # full rowsums of K' incl diag
    Tp = np.zeros((D, D), dtype=np.float64)
    cdiag = float(C32)
    for r in range(NCORES):
        res = results[r]
        rs = res["rsums"].astype(np.float64)          # [P, NPH*D]
        for ph in range(NPH):
            jb = r if PH_BLK[ph] == 0 else r + 8
            for c in range(D):
                Ap[c, jb * P : (jb + 1) * P] += rs[:, ph * D + c]
        cs = res["cs"].astype(np.float64)             # [120, 256]
        row = 0
        for blk, (jb, S) in enumerate(((r, S0), (r + 8, S1))):
            for sidx in range(8, S // 16):
                vals = cs[row].reshape(2, D, GJ)      # (g-sub, c, i)
                row += 1
                for k in range(2):
                    gl = 2 * sidx + k
                    i0 = (128 * jb + gl * GJ) % N
                    Ap[:, i0 : i0 + GJ] += vals[k]
        gm = res["gram"].astype(np.float64)           # [P, 4*256]
        diag = (gm[:, 0:CH] + gm[:, 256 + CH : 512]).reshape(D, GJ, D, GJ)
        offd = (gm[:, 512 : 512 + CH] + gm[:, 768 + CH :]).reshape(
            D, GJ, D, GJ
        )
        Tp += np.einsum("aibi->ab", diag) + 2.0 * np.einsum("aibi->ab", offd)

    A = (SQPI / 2.0) * (Ap - cdiag)             # undo 2/sqrt(pi), remove diag
    T = (np.pi / 4.0) * (Tp - N * cdiag * cdiag)
    S = A.sum(axis=1)
    Dm = A @ A.T
    c0 = 1.0 / (N * (N - 3))
    hsic = c0 * (
        T + np.outer(S, S) / ((N - 1.0) * (N - 2.0)) - (2.0 / (N - 2.0)) * Dm
    )
    iu = np.triu_indices(D, 1)
    return np.float32(np.sum(hsic[iu] ** 2))


def run_spmd(in_maps, **kwargs):
    from concourse import bass_utils

    nc = _get_nc()
    return bass_utils.run_bass_kernel_spmd(
        nc, in_maps, core_ids=list(range(NCORES)), **kwargs
    )


def kernel(X):
    X = np.ascontiguousarray(np.asarray(X, dtype=np.float32))
    in_maps = _make_in_maps(X)
    res = run_spmd(in_maps)
    return _combine(res.results)


# revision 18
# speedup vs baseline: 1.0646x; 1.0646x over previous
"""HSIC loss kernel for Trainium2, 8 NeuronCores.

Math: for each feature column c of X [2048, 16], K_c = rbf kernel matrix
(zero diag). Output = sum over feature pairs a<b of squared unbiased-HSIC
combination of T[a,b]=sum(K_a*K_b), rowsums A, total sums S.

Device strategy (SPMD, symmetric-triangle sharding, all-f32 precision):
  Block triangle coverage: core r owns j-blocks jb=r (i-span 1152) and
  jb=r+8 (i-span 1024); host rotates each core's input planes by 128*r so
  every core compiles the SAME instruction stream.
     K' = Derivative_Erf(y_i - y_j),  y = fp16(sqrt(g_c) * x)
  One ACT per (block, phase, c) with the phase i-span as free dim, f32r
  output into a blocked [(i//8), c, i%8] ktile (full f32 precision).
  f32r matmuls hit the 1-cycle/column mode only at moving dim 256, so
  gram matmuls use 2-chunk rhs strips [128,256] with even/odd PSUM
  accumulator classes (left/right output halves are the useful ones),
  and column-sum matmuls use the same 256-strips with a sliding one-hot
  [128,7] weight slice so 4 consecutive strips land on PSUM partitions
  0..3 of one tile and drain with one cheap 4-partition DVE copy.
  Rowsums: ACT accum_out on one phase (keeps Scalar near pure-ACTIVATE
  occupancy) + DVE 4D-AP reduces for the rest.  Host assembles A, T in
  float64 and finishes the tiny [16,16] HSIC.
"""

import sys
import numpy as np

if "/opt/trn_rl_repo" not in sys.path:
    sys.path.insert(0, "/opt/trn_rl_repo")

N = 2048
D = 16
P = 128
NCORES = 8
GJ = 8                    # i's packed per gram chunk col-group -> D*GJ = 128
CH = D * GJ               # 128
S0, S1 = 1152, 1024       # i-spans of block 0 (jb=r) and block 1 (jb=r+8)
OFF1 = 1024               # plane offset of block 1
PHASES = ((0, 384), (384, 384), (768, 384), (0, 384), (384, 320), (704, 320))
PH_BLK = (0, 0, 0, 1, 1, 1)
ACC_PH = ()               # phases whose rowsums come from ACT accum_out
NPH = len(PHASES)
SQPI = float(np.sqrt(np.pi))
C32 = np.float32(2.0 / np.sqrt(np.pi))      # DErf(0) table value on device
NCSROW = (S0 // 16 - 8) + (S1 // 16 - 8)    # 120 off-diag colsum strips

_NC_CACHE = {}


def _patch_tile_drain():
    """Walrus in this container accepts only 1 sync-wait per instruction.
    Tile routinely attaches several. Hoist extra waits onto single-wait NoOp
    carriers emitted just before the instruction on the same engine, and
    split the tail drain's per-engine waits the same way."""
    import concourse.mybir as mybir
    import concourse.tile as tile_mod
    from concourse.vector_clock import ScopedClock, VectorClock

    if getattr(tile_mod.TileContext, "_drain_patched", False):
        return

    orig_add = tile_mod.TileContext._add_instruction
    counter = [0]

    def _add_instruction(self, inst):
        si = inst.sync_info
        if si is not None and si.on_wait is not None and len(si.on_wait) > 1:
            waits = list(si.on_wait)
            for w in waits[:-1]:
                counter[0] += 1
                carrier = mybir.InstNoOp(name=f"waitc-{counter[0]}")
                carrier.engine = inst.engine
                carrier.sync_info = mybir.SyncInfo(on_wait=[w], on_update=[])
                orig_add(self, carrier)
            inst.sync_info = mybir.SyncInfo(
                on_wait=[waits[-1]], on_update=list(si.on_update or [])
            )
        orig_add(self, inst)

    def _drain_and_barrier(self, tick_clock, wait_clock):
        vec = list(tick_clock.global_clock)
        for i, v in enumerate(vec):
            if v <= 0:
                continue
            sub = [v if j == i else 0 for j in range(len(vec))]
            carrier = self.nc.sync.nop(nofuse=True)
            wait_clock.add_sem_waits(
                carrier.ins, ScopedClock({None: VectorClock(sub)})
            )
        self.nc.sync.drain()
        self.nc.all_engine_barrier()
        popped = self.nc._tile_sem_poison_stack.pop()
        assert popped is self._sem_poison
        self.nc.clear_and_free_semaphores(list(self.sems.allocated().values()))
        self.nc.all_engine_barrier()

    tile_mod.TileContext._add_instruction = _add_instruction
    tile_mod.TileContext._drain_and_barrier = _drain_and_barrier
    tile_mod.TileContext._drain_patched = True


def _build_nc():
    import concourse.bass as bass
    import concourse.mybir as mybir
    from concourse.tile import TileContext

    _patch_tile_drain()

    f32 = mybir.dt.float32
    f32r = mybir.dt.float32r
    f16 = mybir.dt.float16

    nc = bass.Bass("TRN2")
    xrep_d = nc.dram_tensor("xrep", [P, D * (S0 + S1)], f16, kind="ExternalInput")
    bias_d = nc.dram_tensor("bias_t", [P, 2 * D], f32, kind="ExternalInput")
    z_d = nc.dram_tensor("z_t", [P, 63], f32r, kind="ExternalInput")
    rs_d = nc.dram_tensor("rsums", [P, NPH * D], f32, kind="ExternalOutput")
    gram_d = nc.dram_tensor("gram", [P, 4 * 256], f32, kind="ExternalOutput")
    cs_d = nc.dram_tensor("cs", [NCSROW, 256], f32, kind="ExternalOutput")

    # gram accumulation counts per class (diag/offd x even/odd g)
    total_uses = [0, 0, 0, 0]
    for S in (S0, S1):
        for g in range(S // GJ):
            total_uses[(0 if g < 16 else 2) + (g % 2)] += 1

    with TileContext(nc) as tc:
        with (
            tc.tile_pool(name="const", bufs=1) as cpool,
            tc.tile_pool(name="xp", bufs=6) as xpool,
            tc.tile_pool(name="csb", bufs=2) as csbpool,
            tc.tile_pool(name="gps", bufs=1, space="PSUM") as gpool,
            tc.tile_pool(name="cps", bufs=3, space="PSUM") as cspool,
            tc.tile_pool(name="wm", bufs=1, space="PSUM") as wpool,
        ):
            bias_sb = cpool.tile([P, 2 * D], f32)
            zt = cpool.tile([P, 63], f32r)
            rsums = cpool.tile([P, NPH * D], f32)
            scratch = cpool.tile([P, 1], f32)
            gsb = cpool.tile([P, 4 * 256], f32)
            kt0 = cpool.tile([P, S0 * D], f32r)
            kt1 = cpool.tile([P, S1 * D], f32r)

            # dummy ACT on a framework const AP: pulls the Derivative_Erf
            # table load forward so it overlaps the first input DMAs
            nc.scalar.activation(
                out=scratch[:],
                in_=nc.const_aps.tensor(0.0, (P, 1), f32),
                func=mybir.ActivationFunctionType.Derivative_Erf,
            )
            nc.sync.dma_start(bias_sb[:], bias_d[:])

            grams = [gpool.tile([P, 256], f32, name=f"gram{t}") for t in range(4)]
            use_ct = [0, 0, 0, 0]
            # colsum psum uses: (n_strips, cs_d row base) per 32-strip group
            cs_uses = [(32, 0), (32, 32), (32, 64), (24, 96)]
            cs_use = 0
            cs_r = 0
            cs_tile = None

            def drain_grams(ts):
                for t in ts:
                    nc.vector.tensor_copy(
                        gsb[:, t * 256 : (t + 1) * 256], grams[t][:]
                    )

            wps = wpool.tile([1, 256], f32, name="warm")
            warm_batch = 0

            xoff = 0             # running DRAM column offset of xrep
            for ph, (g0i, Si) in enumerate(PHASES):
                blk = PH_BLK[ph]
                kt = kt0 if blk == 0 else kt1
                k4 = kt[:].rearrange("p (g c i) -> p g c i", c=D, i=GJ)
                g0 = g0i // GJ
                Gh = Si // GJ
                # input: batched DMAs of feature-strips. Phase 0 splits the
                # first batch in half so the first ACT starts sooner.
                batches = [2, 2, 4, 4, 4] if ph == 0 else [4, 4, 4, 4]
                c = 0
                for bi, nb in enumerate(batches):
                    xq = xpool.tile([P, nb * Si], f16, name="xq")
                    nc.sync.dma_start(
                        xq[:], xrep_d[:, xoff : xoff + nb * Si]
                    )
                    xoff += nb * Si
                    if ph == 0 and bi == 0:
                        # zt load can trail the first input batch
                        nc.sync.dma_start(zt[:], z_d[:])
                    if warm_batch < 7:
                        # dummy f16 matmuls tied to early input batches:
                        # keeps the PE HAM window busy during phase-0 ACTs
                        # (spaced by DMA arrivals) so real matmuls start warm
                        warm_batch += 1
                        for _ in range(2):
                            nc.tensor.matmul(
                                wps[:],
                                lhsT=xq[:, 0:1],
                                rhs=xq[:, 0:256],
                                start=True,
                                stop=True,
                            )
                    for k in range(nb):
                        acc = (
                            rsums[:, ph * D + c : ph * D + c + 1]
                            if ph in ACC_PH
                            else None
                        )
                        nc.scalar.activation(
                            out=k4[:, g0 : g0 + Gh, c, :],
                            in_=xq[:, k * Si : (k + 1) * Si],
                            func=mybir.ActivationFunctionType.Derivative_Erf,
                            bias=bias_sb[:, blk * D + c : blk * D + c + 1],
                            accum_out=acc,
                        )
                        c += 1
                if ph not in ACC_PH:
                    # rowsums for this phase on DVE (4D AP, reduce over g,i)
                    k4c = kt[:].rearrange("p (g c i) -> p c g i", c=D, i=GJ)
                    nc.vector.tensor_reduce(
                        out=rsums[:, ph * D : (ph + 1) * D],
                        in_=k4c[:, :, g0 : g0 + Gh, :],
                        axis=mybir.AxisListType.XY,
                        op=mybir.AluOpType.add,
                    )
                # grams + colsums per 2-chunk 256-col strip.  In the last
                # phase emit all colsums first so the final cs drain + DMA
                # overlap the remaining gram matmuls instead of tailing.
                def emit_gram(s):
                    rhs = kt[:, s * 256 : (s + 1) * 256]
                    for g in (2 * s, 2 * s + 1):
                        t = (0 if g < 16 else 2) + (g % 2)
                        nc.tensor.matmul(
                            grams[t][:],
                            lhsT=kt[:, g * CH : (g + 1) * CH],
                            rhs=rhs,
                            start=(use_ct[t] == 0),
                            stop=(use_ct[t] == total_uses[t] - 1),
                        )
                        use_ct[t] += 1
                        if use_ct[t] == total_uses[t] and t == 1:
                            drain_grams((0, 1))   # diag classes done early
                            nc.sync.dma_start(
                                gram_d[:, 0:512], gsb[:, 0:512]
                            )

                def emit_colsum(s):
                    nonlocal cs_use, cs_r, cs_tile
                    if s < 8:
                        return  # diag strip: colsums not needed
                    nuse, rowbase = cs_uses[cs_use]
                    r = cs_r
                    if r == 0:
                        cs_tile = cspool.tile([32, 256], f32, name="cs")
                    # lhsT = zt[:, 31-r : 63-r] puts the ones column at row r
                    nc.tensor.matmul(
                        cs_tile[0:32, :],
                        lhsT=zt[:, 31 - r : 63 - r],
                        rhs=kt[:, s * 256 : (s + 1) * 256],
                        start=(r == 0),
                        stop=(r == nuse - 1),
                    )
                    cs_r += 1
                    if cs_r == nuse:
                        cs_r = 0
                        cs_use += 1
                        stage = csbpool.tile([32, 256], f32, name="csb")
                        nc.vector.tensor_copy(
                            stage[0:nuse, :], cs_tile[0:nuse, :]
                        )
                        nc.sync.dma_start(
                            cs_d[rowbase : rowbase + nuse, :], stage[0:nuse, :]
                        )

                strips = range(g0i // 16, (g0i + Si) // 16)
                if ph == NPH - 1:
                    for s in strips:
                        emit_colsum(s)
                    for s in strips:
                        emit_gram(s)
                else:
                    for s in strips:
                        emit_gram(s)
                        emit_colsum(s)

            nc.sync.dma_start(rs_d[:], rsums[:])
            drain_grams((2, 3))
            nc.sync.dma_start(gram_d[:, 512:], gsb[:, 512:])
    return nc


def _get_nc():
    if "nc" not in _NC_CACHE:
        _NC_CACHE["nc"] = _build_nc()
    return _NC_CACHE["nc"]


def _make_in_maps(X):
    Xd = X.astype(np.float64)
    meanD = 2.0 * (np.mean(Xd * Xd, axis=0) - np.mean(Xd, axis=0) ** 2)  # [D]
    g = 1.0 / (2.0 * meanD)                # gamma = 1/(2*sigma^2)
    s = np.sqrt(g).astype(np.float32)      # sqrt(gamma) per column

    # Device sees fp16-rounded prescaled samples y = fp16(s_c * x); build the
    # bias from the same rounded values so the kernel diagonal is DErf(0).
    Y16 = (X.astype(np.float32) * s[None, :]).astype(np.float16)   # [N, D]

    # one-hot weight for colsum matmuls: col 31 ones, rest zero
    z_t = np.zeros((P, 63), np.float32)
    z_t[:, 31] = 1.0

    in_maps = []
    for r in range(NCORES):
        cols = []
        for ph, (g0i, Si) in enumerate(PHASES):
            blk = PH_BLK[ph]
            base = 128 * r + (OFF1 if blk else 0) + g0i
            idx = (base + np.arange(Si)) % N
            for c in range(D):
                cols.append(Y16[idx, c])
        xrow = np.concatenate(cols)                       # [D*(S0+S1)] f16
        xrep = np.ascontiguousarray(
            np.broadcast_to(xrow[None, :], (P, xrow.shape[0]))
        )
        bias = np.empty((P, 2 * D), np.float32)
        for blk, jb in enumerate((r, r + 8)):
            yj = Y16[jb * P : (jb + 1) * P, :].astype(np.float32)  # [P, D]
            bias[:, blk * D : (blk + 1) * D] = -yj
        in_maps.append({"xrep": xrep, "bias_t": bias, "z_t": z_t})
    return in_maps


def _combine(results):
    # Device K' = (2/sqrt(pi)) * K, exact f32 everywhere.
    Ap = np.zeros((D, N), dtype=np.float64)   # full rowsums of K' incl diag
    Tp = np.zeros((D, D), dtype=np.float64)
    cdiag = float(C32)
    for r in range(NCORES):
        res = results[r]
        rs = res["rsums"].astype(np.float64)          # [P, NPH*D]
        for ph in range(NPH):
            jb = r if PH_BLK[ph] == 0 else r + 8
            for c in range(D):
                Ap[c, jb * P : (jb + 1) * P] += rs[:, ph * D + c]
        cs = res["cs"].astype(np.float64)             # [120, 256]
        row = 0
        for blk, (jb, S) in enumerate(((r, S0), (r + 8, S1))):
            for sidx in range(8, S // 16):
                vals = cs[row].reshape(2, D, GJ)      # (g-sub, c, i)
                row += 1
                for k in range(2):
                    gl = 2 * sidx + k
                    i0 = (128 * jb + gl * GJ) % N
                    Ap[:, i0 : i0 + GJ] += vals[k]
        gm = res["gram"].astype(np.float64)           # [P, 4*256]
        diag = (gm[:, 0:CH] + gm[:, 256 + CH : 512]).reshape(D, GJ, D, GJ)
        offd = (gm[:, 512 : 512 + CH] + gm[:, 768 + CH :]).reshape(
            D, GJ, D, GJ
        )
        Tp += np.einsum("aibi->ab", diag) + 2.0 * np.einsum("aibi->ab", offd)

    A = (SQPI / 2.0) * (Ap - cdiag)             # undo 2/sqrt(pi), remove diag
    T = (np.pi / 4.0) * (Tp - N * cdiag * cdiag)
    S = A.sum(axis=1)
    Dm = A @ A.T
    c0 = 1.0 / (N * (N - 3))
    hsic = c0 * (
        T + np.outer(S, S) / ((N - 1.0) * (N - 2.0)) - (2.0 / (N - 2.0)) * Dm
    )
    iu = np.triu_indices(D, 1)
    return np.float32(np.sum(hsic[iu] ** 2))


def run_spmd(in_maps, **kwargs):
    from concourse import bass_utils

    nc = _get_nc()
    return bass_utils.run_bass_kernel_spmd(
        nc, in_maps, core_ids=list(range(NCORES)), **kwargs
    )


def kernel(X):
    X = np.ascontiguousarray(np.asarray(X, dtype=np.float32))
    in_maps = _make_in_maps(X)
    res = run_spmd(in_maps)
    return _combine(res.results)
